# revision 17
# baseline (speedup 1.0000x reference)
"""Contrastive loss (InfoNCE, diagonal labels) Trainium2 kernel.

loss = -mean_i log_softmax(E_n @ E_n.T / T)[i, i],  E_n = L2-normalized rows.

Rewritten per-row as  loss_i = log( sum_j exp((s_ij - s_ii) / T) )  which is
exact (s_ii is the row max since rows are unit vectors) and numerically
stable. The softmax-shift bias is derived from the PE's own accumulation of
the diagonal (see ssb below), making the diagonal term exactly 1.

Sharding: row-parallel over 8 cores. Each core receives the FULL (key-side)
operand plus its own 2048-row slice, computes its [2048, 16384] logits block
tile-by-tile (never materialized), and outputs its 2048 per-row losses; the
host takes the mean. No collectives needed.

Host prep (O(N*D), 0.4% of total FLOPs): L2-normalize rows, scale by QS=16,
cast bf16, lay out transposed as [2, 128, N] (two 128-d k-tiles). Device:
  - streams the key side in 2048-col chunks, casting bf16 -> fp8e4 on the
    otherwise-idle GPSIMD engine,
  - one DoubleRow fp8 matmul per PSUM bank (K=256 double-pumped, 0.5
    cyc/row) -> [128, 2048] tiles,
  - ScalarE exp(ascale*x + bias_i) with fused accum_out row-sums for most
    tiles; a fixed subset of row-groups (OFF_G) is handled by the Vector
    engine instead via a round-to-int16 / bitcast-bf16 2^t evaluation, to
    split the N^2 exponential work across two engines.
"""

import sys

sys.path.insert(0, "/opt/trn_rl_repo")

from contextlib import ExitStack

import ml_dtypes
import numpy as np

import concourse.bass as bass
import concourse.tile as tile
from concourse import bacc, masks, mybir
from concourse.bass_utils import run_bass_kernel_spmd

# The act-table insertion pass greedily picks the first table-set containing
# each activation function; keep Exp+Ln served from their combined set so the
# single explicit table load covers both.
_orig_get_act_tables = bacc.get_activation_tables


def _combined_exp_ln_tables(arch):
    tabs = _orig_get_act_tables(arch)
    both = mybir.ActivationFunctionType.Exp, mybir.ActivationFunctionType.Ln
    out = {}
    for name, fns in tabs.items():
        if name != "natural_log_exp_and_others" and all(f in fns for f in both):
            name_keep = False
        else:
            name_keep = name == "natural_log_exp_and_others"
        if not name_keep:
            fns = {f for f in fns if f not in both}
        out[name] = fns
    return out


bacc.get_activation_tables = _combined_exp_ln_tables

N = 16384  # total rows
D = 256  # embedding dim
P = 128  # partitions
CORES = 8
R = N // CORES  # rows per core = 2048
NCH = 8  # key-side 2048-col chunks
GR = R // P  # 16 own row-groups
NJ = 512  # matmul free dim (one PSUM bank, fp32)
JB = 4  # PSUM banks per exp tile -> free dim 2048
JGRP = N // (JB * NJ)  # 8 j-groups; j-group jj consumes key chunk jj
TEMP = 0.07
QS = 16.0  # operand scale; psum values are QS^2 * s_ij
SCALE = float(1.0 / TEMP)
# fp8 e4m3 round-to-nearest of ~N(0,1) values is a slight multiplicative
# shrinkage q ~= (1+C8)*v (C8 = E[v*err]/E[v^2], a quantization-law constant
# for this distribution, seed-independent). Both operands shrink, so the psum
# carries a (1+C8)^2 gain; dividing the activation scale by it removes a
# +3%-ish systematic bias on the off-diagonal exp sums. GAM2 is the fitted
# second-order residual of the same model (also seed-stable).
C8 = -0.0011023823
GAM2 = -5.05e-4
ASCALE = SCALE / (QS * QS) / (1.0 + C8) ** 2 * (1.0 + GAM2)

f32 = mybir.dt.float32
bf16 = mybir.dt.bfloat16
fp8 = mybir.dt.float8e4
i16 = mybir.dt.int16
MULT = mybir.AluOpType.mult
ADD = mybir.AluOpType.add
ISGT = mybir.AluOpType.is_gt
EXP = mybir.ActivationFunctionType.Exp
LN = mybir.ActivationFunctionType.Ln
AXX = mybir.AxisListType.X
DR = mybir.MatmulPerfMode.DoubleRow

# --- DVE exp2-bitcast offload ---------------------------------------------
# Tiles (g, jj) with g in OFF_G skip the ScalarE exp: DVE computes
# bits = round(A*psum + B_p) as int16, bitcasts to bf16 (2^t with a linear
# mantissa chord), and row-reduces. CAL centers the one-sided (1+f)/2^f
# chord error (mean +4.2%) to ~zero under the exp-weighted f-distribution;
# the diagonal (arg exactly 0 thanks to the PE-matched ssb) maps to the
# known constant V_DIAG and is restored to exactly 1.0 via an is_gt mask on
# the per-(g,jj) partial sums (off-diagonal partials are ~2e-3).
OFF_G = ()  # row-groups whose 8 j-group tiles go to DVE
L2E = 1.4426950408889634
CAL = -7.2
V_DIAG = 0.97265625  # bitcast(round(16256 + CAL)) = bits 16249


def build_program():
    nc = bacc.Bacc("TRN2", target_bir_lowering=False, debug=False, num_devices=CORES)
    # host-prepped key-side operand, transposed: [ktile, d, col] bf16 at QS
    ebT_in = nc.dram_tensor("ebT", [2, P, N], bf16, kind="ExternalInput").ap()
    rowsT_in = nc.dram_tensor("rowsT", [2, P, R], bf16, kind="ExternalInput").ap()
    out = nc.dram_tensor("out_rows", [R], f32, kind="ExternalOutput").ap()

    with tile.TileContext(nc) as tc:
        with ExitStack() as ctx:
            persist = ctx.enter_context(tc.tile_pool(name="persist", bufs=1))
            loads = ctx.enter_context(tc.tile_pool(name="loads", bufs=3))
            psum = ctx.enter_context(
                tc.tile_pool(name="psum", bufs=2, space=bass.MemorySpace.PSUM)
            )
            dumps = ctx.enter_context(tc.tile_pool(name="dumps", bufs=2))
            small = ctx.enter_context(tc.tile_pool(name="small", bufs=1))

            embT8 = [
                persist.tile([P, 2, JB * NJ], fp8, name=f"embT8_{t}")
                for t in range(NCH)
            ]
            rowsT8 = persist.tile([P, 2, R], fp8, name="rowsT8")
            ssb = persist.tile([P, GR], f32, name="ssb")
            bias = persist.tile([P, GR], f32, name="bias")
            boff = persist.tile([P, GR], f32, name="boff")
            sp_all = persist.tile([P, GR * JGRP], f32, name="sp_all")
            s_col = persist.tile([P, GR], f32, name="s_col")
            lout = persist.tile([P, GR], f32, name="lout")

            ebT_g = ebT_in.rearrange("k p n -> p k n")
            rowsT_g = rowsT_in.rearrange("k p n -> p k n")

            def load_chunk(t):
                etb = loads.tile([P, 2, JB * NJ], bf16, tag="etb")
                nc.sync.dma_start(
                    etb[:], ebT_g[:, :, t * JB * NJ : (t + 1) * JB * NJ]
                )
                for kc in range(2):
                    nc.gpsimd.tensor_copy(embT8[t][:, kc], etb[:, kc])

            # own rows: load transposed bf16, cast to fp8 on Pool
            rtb = loads.tile([P, 2, R], bf16, tag="rtb")
            nc.sync.dma_start(rtb[:], rowsT_g)
            for kc in range(2):
                nc.gpsimd.tensor_copy(rowsT8[:, kc], rtb[:, kc])

            # ssb must equal the main-loop diagonal BITWISE: the PE DoubleRow
            # accumulator is reduced-precision (~2^-13 rel, truncating), so an
            # f32 DVE/ACT sum of squares is off by ~1e-2 absolute on 256 — a
            # percent-level loss error after the x66 1/eps amplification. So
            # run the same dot through the same PE circuit (tiny per-group
            # gram matmuls) and pick out the diagonal with an identity mask.
            ident = persist.tile([P, P], f32, name="ident")
            masks.make_identity(nc, ident[:])
            dgd = dumps.tile([P, P], f32, tag="dgd")
            for g in range(GR):
                pmg = psum.tile([P, P], f32, tag="ps")
                nc.tensor.matmul(
                    pmg[:],
                    rowsT8[:, :, g * P : (g + 1) * P],
                    rowsT8[:, :, g * P : (g + 1) * P],
                    start=True,
                    stop=True,
                    perf_mode=DR,
                )
                nc.vector.scalar_tensor_tensor(
                    out=dgd[:],
                    in0=pmg[:],
                    scalar=1.0,
                    in1=ident[:],
                    op0=MULT,
                    op1=MULT,
                    accum_out=ssb[:, g : g + 1],
                )
            nc.vector.tensor_scalar_mul(bias[:], ssb[:], -ASCALE)
            A_OFF = float(128.0 * L2E * ASCALE)
            nc.vector.tensor_scalar(
                out=boff[:],
                in0=ssb[:],
                scalar1=-A_OFF,
                scalar2=16256.0 + CAL,
                op0=MULT,
                op1=ADD,
            )
            load_chunk(0)

            # main: j-group jj uses only key chunk jj, prepared one j-group
            # ahead so the key-side streaming overlaps compute.
            for jj in range(JGRP):
                for g in range(GR):
                    if g == 1 and jj + 1 < JGRP:
                        load_chunk(jj + 1)
                    pm = psum.tile([P, JB * NJ], f32, tag="ps")
                    for jb in range(JB):
                        nc.tensor.matmul(
                            pm[:, jb * NJ : (jb + 1) * NJ],
                            rowsT8[:, :, g * P : (g + 1) * P],
                            embT8[jj][:, :, jb * NJ : (jb + 1) * NJ],
                            start=True,
                            stop=True,
                            perf_mode=DR,
                        )
                    spc = sp_all[:, g * JGRP + jj : g * JGRP + jj + 1]
                    if g in OFF_G:
                        ti = dumps.tile([P, JB * NJ], i16, tag="ti")
                        nc.vector.tensor_scalar(
                            out=ti[:],
                            in0=pm[:],
                            scalar1=A_OFF,
                            scalar2=boff[:, g : g + 1],
                            op0=MULT,
                            op1=ADD,
                        )
                        # tree-reduce: bf16 halving adds run at the DVE 2x
                        # perf mode, unlike InstTensorReduce which has none
                        tb = ti[:].bitcast(bf16)
                        scr = dumps.tile([P, 1792], bf16, tag="tr")
                        nc.vector.tensor_tensor(
                            out=scr[:, 0:1024], in0=tb[:, 0:1024],
                            in1=tb[:, 1024:2048], op=ADD,
                        )
                        nc.vector.tensor_tensor(
                            out=scr[:, 1024:1536], in0=scr[:, 0:512],
                            in1=scr[:, 512:1024], op=ADD,
                        )
                        nc.vector.tensor_tensor(
                            out=scr[:, 1536:1792], in0=scr[:, 1024:1280],
                            in1=scr[:, 1280:1536], op=ADD,
                        )
                        nc.vector.reduce_sum(spc, scr[:, 1536:1792], axis=AXX)
                    else:
                        # in-place exp over the PSUM tile: all big operands in
                        # PSUM keeps the ACT access penalty at the PSUM rate
                        # and drops the SBUF dump tile entirely
                        nc.scalar.activation(
                            pm[:],
                            pm[:],
                            EXP,
                            bias=bias[:, g : g + 1],
                            scale=ASCALE,
                            accum_out=spc,
                        )
            # restore the diagonal term (known constant under the bitcast
            # exp2) to exactly 1.0 in the offloaded partials
            for g in OFF_G:
                srun = sp_all[:, g * JGRP : (g + 1) * JGRP]
                msk = small.tile([P, JGRP], f32, tag="msk", bufs=4)
                nc.vector.tensor_scalar(
                    out=msk[:], in0=srun, scalar1=0.5, scalar2=None, op0=ISGT
                )
                nc.vector.scalar_tensor_tensor(
                    out=srun,
                    in0=msk[:],
                    scalar=1.0 - V_DIAG,
                    in1=srun,
                    op0=MULT,
                    op1=ADD,
                )
            for g in range(GR):
                nc.vector.reduce_sum(
                    s_col[:, g : g + 1],
                    sp_all[:, g * JGRP : (g + 1) * JGRP],
                    axis=AXX,
                )
            nc.scalar.activation(lout[:], s_col[:], LN)
            nc.sync.dma_start(out.rearrange("(u p) -> p u", p=P), lout[:])

    nc.compile()
    return nc


def _host_prep(embeddings: np.ndarray) -> np.ndarray:
    """L2-normalize rows, scale by QS, cast bf16, transpose to [2, 128, N]."""
    e = embeddings.astype(np.float32)
    ss = (e * e).sum(axis=1)
    rinv = (QS / np.sqrt(ss)).astype(np.float32)
    nrm = (e * rinv[:, None]).astype(ml_dtypes.bfloat16)  # [N, D]
    return np.ascontiguousarray(nrm.T.reshape(2, P, N))


def run_cores(embeddings: np.ndarray, trace: bool = False):
    nc = build_program()
    ebT = _host_prep(embeddings)
    in_maps = [
        {
            "ebT": ebT,
            "rowsT": np.ascontiguousarray(ebT[:, :, c * R : (c + 1) * R]),
        }
        for c in range(CORES)
    ]
    return run_bass_kernel_spmd(nc, in_maps, list(range(CORES)), trace=trace)


def kernel(embeddings: np.ndarray) -> np.ndarray:
    embeddings = np.ascontiguousarray(np.asarray(embeddings, dtype=np.float32))
    assert embeddings.shape == (N, D)
    res = run_cores(embeddings)
    vals = np.concatenate([res.results[c]["out_rows"] for c in range(CORES)])
    return np.float32(vals.mean())


# revision 20
# speedup vs baseline: 1.2511x; 1.2511x over previous
"""Contrastive loss (InfoNCE, diagonal labels) Trainium2 kernel.

loss = -mean_i log_softmax(E_n @ E_n.T / T)[i, i],  E_n = L2-normalized rows.

Rewritten per-row as  loss_i = log( sum_j exp((s_ij - s_ii) / T) )  which is
exact (s_ii is the row max since rows are unit vectors) and numerically
stable. The softmax-shift bias is derived from the PE's own accumulation of
the diagonal (see ssb below), making the diagonal term exactly 1.

Sharding: row-parallel over 8 cores. Each core receives the FULL (key-side)
operand plus its own 2048-row slice, computes its [2048, 16384] logits block
tile-by-tile (never materialized), and outputs its 2048 per-row losses; the
host takes the mean. No collectives needed.

Host prep (O(N*D), 0.4% of total FLOPs): L2-normalize rows, scale by QS=16,
cast bf16, lay out transposed as [2, 128, N] (two 128-d k-tiles). Device:
  - streams the key side in 2048-col chunks, casting bf16 -> fp8e4 on the
    otherwise-idle GPSIMD engine,
  - one DoubleRow fp8 matmul per PSUM bank (K=256 double-pumped, 0.5
    cyc/row) -> [128, 2048] tiles,
  - ScalarE exp(ascale*x + bias_i) with fused accum_out row-sums for most
    tiles; a fixed subset of row-groups (OFF_G) is handled by the Vector
    engine instead via a round-to-int16 / bitcast-bf16 2^t evaluation, to
    split the N^2 exponential work across two engines.
"""

import sys

sys.path.insert(0, "/opt/trn_rl_repo")

from contextlib import ExitStack

import ml_dtypes
import numpy as np

import concourse.bass as bass
import concourse.tile as tile
from concourse import bacc, masks, mybir
from concourse.bass_utils import run_bass_kernel_spmd

# The act-table insertion pass greedily picks the first table-set containing
# each activation function; keep Exp+Ln served from their combined set so the
# single explicit table load covers both.
_orig_get_act_tables = bacc.get_activation_tables


def _combined_exp_ln_tables(arch):
    tabs = _orig_get_act_tables(arch)
    both = mybir.ActivationFunctionType.Exp, mybir.ActivationFunctionType.Ln
    out = {}
    for name, fns in tabs.items():
        if name != "natural_log_exp_and_others" and all(f in fns for f in both):
            name_keep = False
        else:
            name_keep = name == "natural_log_exp_and_others"
        if not name_keep:
            fns = {f for f in fns if f not in both}
        out[name] = fns
    return out


bacc.get_activation_tables = _combined_exp_ln_tables

N = 16384  # total rows
D = 256  # embedding dim
P = 128  # partitions
CORES = 8
R = N // CORES  # rows per core = 2048
NCH = 8  # key-side 2048-col chunks
GR = R // P  # 16 own row-groups
NJ = 512  # matmul free dim (one PSUM bank, fp32)
JB = 4  # PSUM banks per exp tile -> free dim 2048
JGRP = N // (JB * NJ)  # 8 j-groups; j-group jj consumes key chunk jj
TEMP = 0.07
QS = 16.0  # operand scale; psum values are QS^2 * s_ij
SCALE = float(1.0 / TEMP)
# fp8 e4m3 round-to-nearest of ~N(0,1) values is a slight multiplicative
# shrinkage q ~= (1+C8)*v (C8 = E[v*err]/E[v^2], a quantization-law constant
# for this distribution, seed-independent). Both operands shrink, so the psum
# carries a (1+C8)^2 gain; dividing the activation scale by it removes a
# +3%-ish systematic bias on the off-diagonal exp sums. GAM2 is the fitted
# second-order residual of the same model (also seed-stable).
C8 = -0.0011023823
GAM2 = -5.05e-4
ASCALE = SCALE / (QS * QS) / (1.0 + C8) ** 2 * (1.0 + GAM2)

f32 = mybir.dt.float32
bf16 = mybir.dt.bfloat16
fp8 = mybir.dt.float8e4
i16 = mybir.dt.int16
MULT = mybir.AluOpType.mult
ADD = mybir.AluOpType.add
ISGT = mybir.AluOpType.is_gt
EXP = mybir.ActivationFunctionType.Exp
LN = mybir.ActivationFunctionType.Ln
AXX = mybir.AxisListType.X
DR = mybir.MatmulPerfMode.DoubleRow

# --- DVE exp2-bitcast offload ---------------------------------------------
# Tiles (g, jj) with g in OFF_G skip the ScalarE exp: DVE computes
# bits = round(A*psum + B_p) as int16, bitcasts to bf16 (2^t with a linear
# mantissa chord), and row-reduces. CAL centers the one-sided (1+f)/2^f
# chord error (mean +4.2%) to ~zero under the exp-weighted f-distribution;
# the diagonal (arg exactly 0 thanks to the PE-matched ssb) maps to the
# known constant V_DIAG and is restored to exactly 1.0 via an is_gt mask on
# the per-(g,jj) partial sums (off-diagonal partials are ~2e-3).
OFF_G = (1, 4, 7, 10, 13)  # row-groups whose 8 j-group tiles go to DVE
L2E = 1.4426950408889634
CAL = -7.2
V_DIAG = 0.97265625  # bitcast(round(16256 + CAL)) = bits 16249


def build_program():
    nc = bacc.Bacc("TRN2", target_bir_lowering=False, debug=False, num_devices=CORES)
    # host-prepped key-side operand, transposed: [ktile, d, col] bf16 at QS
    ebT_in = nc.dram_tensor("ebT", [2, P, N], bf16, kind="ExternalInput").ap()
    rowsT_in = nc.dram_tensor("rowsT", [2, P, R], bf16, kind="ExternalInput").ap()
    out = nc.dram_tensor("out_rows", [R], f32, kind="ExternalOutput").ap()

    with tile.TileContext(nc) as tc:
        with ExitStack() as ctx:
            persist = ctx.enter_context(tc.tile_pool(name="persist", bufs=1))
            loads = ctx.enter_context(tc.tile_pool(name="loads", bufs=3))
            psum = ctx.enter_context(
                tc.tile_pool(name="psum", bufs=2, space=bass.MemorySpace.PSUM)
            )
            dumps = ctx.enter_context(tc.tile_pool(name="dumps", bufs=2))
            small = ctx.enter_context(tc.tile_pool(name="small", bufs=1))

            embT8 = [
                persist.tile([P, 2, JB * NJ], fp8, name=f"embT8_{t}")
                for t in range(NCH)
            ]
            rowsT8 = persist.tile([P, 2, R], fp8, name="rowsT8")
            ssb = persist.tile([P, GR], f32, name="ssb")
            bias = persist.tile([P, GR], f32, name="bias")
            boff = persist.tile([P, GR], f32, name="boff")
            sp_all = persist.tile([P, GR * JGRP], f32, name="sp_all")
            s_col = persist.tile([P, GR], f32, name="s_col")
            lout = persist.tile([P, GR], f32, name="lout")

            ebT_g = ebT_in.rearrange("k p n -> p k n")
            rowsT_g = rowsT_in.rearrange("k p n -> p k n")

            def load_chunk(t, head=False):
                etb = loads.tile([P, 2, JB * NJ], bf16, tag="etb")
                nc.sync.dma_start(
                    etb[:], ebT_g[:, :, t * JB * NJ : (t + 1) * JB * NJ]
                )
                for kc in range(2):
                    # at the head, split casts across Pool and DVE so the
                    # first chunk isn't serialized behind the rows casts
                    eng = nc.vector if (head and kc == 1) else nc.gpsimd
                    eng.tensor_copy(embT8[t][:, kc], etb[:, kc])

            # own rows: load transposed bf16, cast to fp8 (split engines)
            rtb = loads.tile([P, 2, R], bf16, tag="rtb")
            nc.sync.dma_start(rtb[:], rowsT_g)
            nc.vector.tensor_copy(rowsT8[:, 0], rtb[:, 0])
            nc.gpsimd.tensor_copy(rowsT8[:, 1], rtb[:, 1])

            # ssb must equal the main-loop diagonal BITWISE: the PE DoubleRow
            # accumulator is reduced-precision (~2^-13 rel, truncating), so an
            # f32 DVE/ACT sum of squares is off by ~1e-2 absolute on 256 — a
            # percent-level loss error after the x66 1/eps amplification. So
            # run the same dot through the same PE circuit (tiny per-group
            # gram matmuls) and pick out the diagonal with an identity mask.
            ident = persist.tile([P, P], f32, name="ident")
            masks.make_identity(nc, ident[:])
            dgd = dumps.tile([P, P], f32, tag="dgd")
            for g in range(GR):
                pmg = psum.tile([P, P], f32, tag="ps")
                nc.tensor.matmul(
                    pmg[:],
                    rowsT8[:, :, g * P : (g + 1) * P],
                    rowsT8[:, :, g * P : (g + 1) * P],
                    start=True,
                    stop=True,
                    perf_mode=DR,
                )
                nc.vector.scalar_tensor_tensor(
                    out=dgd[:],
                    in0=pmg[:],
                    scalar=1.0,
                    in1=ident[:],
                    op0=MULT,
                    op1=MULT,
                    accum_out=ssb[:, g : g + 1],
                )
            nc.vector.tensor_scalar_mul(bias[:], ssb[:], -ASCALE)
            A_OFF = float(128.0 * L2E * ASCALE)
            nc.vector.tensor_scalar(
                out=boff[:],
                in0=ssb[:],
                scalar1=-A_OFF,
                scalar2=16256.0 + CAL,
                op0=MULT,
                op1=ADD,
            )
            load_chunk(0, head=True)

            # main: j-group jj uses only key chunk jj, prepared one j-group
            # ahead so the key-side streaming overlaps compute.
            for jj in range(JGRP):
                for g in range(GR):
                    if g == 1 and jj + 1 < JGRP:
                        load_chunk(jj + 1)
                    pm = psum.tile([P, JB * NJ], f32, tag="ps")
                    for jb in range(JB):
                        nc.tensor.matmul(
                            pm[:, jb * NJ : (jb + 1) * NJ],
                            rowsT8[:, :, g * P : (g + 1) * P],
                            embT8[jj][:, :, jb * NJ : (jb + 1) * NJ],
                            start=True,
                            stop=True,
                            perf_mode=DR,
                        )
                    spc = sp_all[:, g * JGRP + jj : g * JGRP + jj + 1]
                    if g in OFF_G:
                        ti = dumps.tile([P, JB * NJ], i16, tag="ti")
                        nc.vector.tensor_scalar(
                            out=ti[:],
                            in0=pm[:],
                            scalar1=A_OFF,
                            scalar2=boff[:, g : g + 1],
                            op0=MULT,
                            op1=ADD,
                        )
                        # tree-reduce: bf16 halving adds run at the DVE 2x
                        # perf mode, unlike InstTensorReduce which has none
                        tb = ti[:].bitcast(bf16)
                        scr = dumps.tile([P, 1792], bf16, tag="tr")
                        nc.vector.tensor_tensor(
                            out=scr[:, 0:1024], in0=tb[:, 0:1024],
                            in1=tb[:, 1024:2048], op=ADD,
                        )
                        nc.vector.tensor_tensor(
                            out=scr[:, 1024:1536], in0=scr[:, 0:512],
                            in1=scr[:, 512:1024], op=ADD,
                        )
                        nc.vector.tensor_tensor(
                            out=scr[:, 1536:1792], in0=scr[:, 1024:1280],
                            in1=scr[:, 1280:1536], op=ADD,
                        )
                        nc.vector.reduce_sum(spc, scr[:, 1536:1792], axis=AXX)
                    else:
                        # in-place exp over the PSUM tile: all big operands in
                        # PSUM keeps the ACT access penalty at the PSUM rate
                        # and drops the SBUF dump tile entirely
                        nc.scalar.activation(
                            pm[:],
                            pm[:],
                            EXP,
                            bias=bias[:, g : g + 1],
                            scale=ASCALE,
                            accum_out=spc,
                        )
            # restore the diagonal term (known constant under the bitcast
            # exp2) to exactly 1.0 in the offloaded partials
            for g in OFF_G:
                srun = sp_all[:, g * JGRP : (g + 1) * JGRP]
                msk = small.tile([P, JGRP], f32, tag="msk", bufs=4)
                nc.vector.tensor_scalar(
                    out=msk[:], in0=srun, scalar1=0.5, scalar2=None, op0=ISGT
                )
                nc.vector.scalar_tensor_tensor(
                    out=srun,
                    in0=msk[:],
                    scalar=1.0 - V_DIAG,
                    in1=srun,
                    op0=MULT,
                    op1=ADD,
                )
            for g in range(GR):
                nc.vector.reduce_sum(
                    s_col[:, g : g + 1],
                    sp_all[:, g * JGRP : (g + 1) * JGRP],
                    axis=AXX,
                )
            nc.scalar.activation(lout[:], s_col[:], LN)
            nc.sync.dma_start(out.rearrange("(u p) -> p u", p=P), lout[:])

    nc.compile()
    return nc


def _host_prep(embeddings: np.ndarray) -> np.ndarray:
    """L2-normalize rows, scale by QS, cast bf16, transpose to [2, 128, N]."""
    e = embeddings.astype(np.float32)
    ss = (e * e).sum(axis=1)
    rinv = (QS / np.sqrt(ss)).astype(np.float32)
    nrm = (e * rinv[:, None]).astype(ml_dtypes.bfloat16)  # [N, D]
    return np.ascontiguousarray(nrm.T.reshape(2, P, N))


def run_cores(embeddings: np.ndarray, trace: bool = False):
    nc = build_program()
    ebT = _host_prep(embeddings)
    in_maps = [
        {
            "ebT": ebT,
            "rowsT": np.ascontiguousarray(ebT[:, :, c * R : (c + 1) * R]),
        }
        for c in range(CORES)
    ]
    return run_bass_kernel_spmd(nc, in_maps, list(range(CORES)), trace=trace)


def kernel(embeddings: np.ndarray) -> np.ndarray:
    embeddings = np.ascontiguousarray(np.asarray(embeddings, dtype=np.float32))
    assert embeddings.shape == (N, D)
    res = run_cores(embeddings)
    vals = np.concatenate([res.results[c]["out_rows"] for c in range(CORES)])
    return np.float32(vals.mean())
